# revision 1
# baseline (speedup 1.0000x reference)
"""Trainium2 Bass kernel v2 for nn_CrossLayerAttention_309237645906.

Reference computation (B=2, SQ=SK=2048, H=2048, NH=16, HD=128, fp32):
    q = hidden @ w_q.T + b_q                     -> [B, NH, SQ, HD]
    scores = mask + scale * q @ k                (k given as [B*NH, HD, SK])
    probs = softmax(scores)                      (fp32)
    out = (probs @ v)                            -> [B, SQ, H]
    y = out @ w_proj.T + b_proj

Sharding: 8 cores = (batch b = c//4) x (query-row subset). For the causal
mask the 512 rows of core c are the strided set {4*i + (c%4)}: every core
then has identical causal structure (q-tile m needs key tiles 0..4m+3), so
one SPMD program is work-exact at 128-granularity: per head, key tile j
only streams query columns [128*(j//4), 512).

All matmuls run in bf16 (1 cycle/row on PE). T-layout throughout
(contraction dim on partitions, no on-device transposes):
    qT[o, i]      = (wq stationary) @ (xT moving)       o-tile == head
    scoresT[j, i] = (k_h tile stationary) @ qT_h         per (head, j-pair)
    scoresT      += ident @ mask_strip   (tiny PE matmul masks the 64-col
                                          diagonal strip; the rest is 0)
    p = exp(scale * scoresT)   (ScalarE reads the PSUM pair directly;
                                DVE does no mask work at all)
    outT_h[d, i] += (v_h tile) @ p ;  Z[1, i] += (ones) @ p
    rb = bcast(1/Z)            (rank-1 PE matmul + DVE copy)
    attnT_h = outT_h * rb      (DVE)
    y[i, o] = (attnT stationary) @ wp moving + b_proj

Early attention heads are interleaved into the q-projection's DMA-limited
stretch (shared scores/psq PSUM pool), the pv/Z consume runs 6 j-pairs
behind scores across head boundaries, and all DMA rides the SP queue in a
hand-scheduled just-in-time order (DMA_ENGINES is one serial FIFO).
GPSIMD is left idle: its Q7 ISA ops and SWDGE queue crash this runtime,
and it cannot touch PSUM. Optional per-stage fp8e4m3 DoubleRow
(0.5 cycles/row) via p1_fp8 / pv_fp8 / p3_fp8 (off by default: measured
rel-err with pv or p3 in fp8 is ~3e-2 > the 2e-2 gate; all-bf16 is 3e-3).
"""

import sys

sys.path.insert(0, "/opt/trn_rl_repo")

import numpy as np

import concourse.bacc as bacc
import concourse.bass as bass
import concourse.mybir as mybir
import concourse.tile as tile
from concourse.bass_utils import run_bass_kernel_spmd

F32 = mybir.dt.float32
BF16 = mybir.dt.bfloat16
FP8 = mybir.dt.float8e4

B, SQ, SK, H, NH = 2, 2048, 2048, 2048, 16
HD = H // NH  # 128
ROWS = 512            # query rows per core
NCORES = 8
KT = H // 128         # 16 contraction tiles for the projections
JT = SK // 128        # 16 key tiles
IT = ROWS // 128      # 4 query 128-tiles per core
SCALE = 1.0 / float(np.sqrt(HD))
WSCALE = 32.0         # host premultiplier for fp8 weight tensors
MULT = mybir.AluOpType.mult
ADD = mybir.AluOpType.add
EXP = mybir.ActivationFunctionType.Exp
IDENT = mybir.ActivationFunctionType.Identity
DR = mybir.MatmulPerfMode.DoubleRow


def build_kernel(causal=True, p1_fp8=False, pv_fp8=False, p3_fp8=False):
    """Build the per-core Bass program."""
    p1_dt = FP8 if p1_fp8 else BF16
    pv_dt = FP8 if pv_fp8 else BF16
    p3_dt = FP8 if p3_fp8 else BF16
    mask_dt = BF16 if causal else F32

    nc = bacc.Bacc()

    xT = nc.dram_tensor("xT", [128, KT, ROWS], p1_dt, kind="ExternalInput")
    wq = nc.dram_tensor("wq", [128, KT, H], p1_dt, kind="ExternalInput")
    bq = nc.dram_tensor("bq", [128, KT, 1], F32, kind="ExternalInput")
    key = nc.dram_tensor("key", [NH, HD, SK], BF16, kind="ExternalInput")
    value = nc.dram_tensor("value", [NH, 128, JT, HD], pv_dt,
                           kind="ExternalInput")
    if causal:
        # 64-col diagonal strips of the (1/scale-premultiplied) mask plus an
        # identity for the strip-accumulate matmul
        maskS = nc.dram_tensor("maskS", [128, JT, 64], BF16,
                               kind="ExternalInput")
        ident = nc.dram_tensor("ident", [128, 128], BF16,
                               kind="ExternalInput")
    else:
        maskT = nc.dram_tensor("maskT", [128, JT, ROWS], mask_dt,
                               kind="ExternalInput")
    wp = nc.dram_tensor("wp", [128, KT, H], p3_dt, kind="ExternalInput")
    bpB = nc.dram_tensor("bpB", [128, H], F32, kind="ExternalInput")
    onesd = nc.dram_tensor("onesd", [128, 2, 1], pv_dt, kind="ExternalInput")
    ones1d = nc.dram_tensor("ones1d", [1, 128], BF16, kind="ExternalInput")
    Y = nc.dram_tensor("Y", [ROWS, H], F32, kind="ExternalOutput")

    with tile.TileContext(nc) as tc:
        with tc.tile_pool(name="res", bufs=1) as res:
            # ---- resident tiles ----
            qT_all = res.tile([128, KT, ROWS], BF16)
            attnT_all = res.tile([128, NH, ROWS], p3_dt)
            if causal:
                maskS_all = res.tile([128, JT, 64], BF16)
                ident_sb = res.tile([128, 128], BF16)
            else:
                maskT_all = res.tile([128, JT, ROWS], mask_dt)
            bq_all = res.tile([128, KT, 1], F32)
            bpB_all = res.tile([128, H], F32)
            ones_sb = res.tile([128, 2, 1], pv_dt)
            ones1_sb = res.tile([1, 128], BF16)
            xT_all = res.tile([128, KT, ROWS], p1_dt)

            # pools living across phases
            kv = tc.alloc_tile_pool(name="kv", bufs=5)
            tp = tc.alloc_tile_pool(name="tp", bufs=4)
            pp = tc.alloc_tile_pool(name="pp", bufs=9)
            sm = tc.alloc_tile_pool(name="sm", bufs=3)
            wqp = tc.alloc_tile_pool(name="wqp", bufs=4)
            wpp = tc.alloc_tile_pool(name="wpp", bufs=16)

            # ---- input streams on the SP DMA queue ----
            # wq streamed in eighths (2 o-tiles each); eighth 0 rides the SP
            # queue right behind the first x chunk so P1 starts ASAP.
            wq_sbs = []

            def fetch_wq(e, eng):
                w_sb = wqp.tile([128, KT, 256], p1_dt, tag="wq",
                                name=f"wq{e}")
                eng.dma_start(w_sb, wq[:, :, 256 * e:256 * (e + 1)])
                wq_sbs.append(w_sb)
                return w_sb

            k_sbs, v_sbs = {}, {}

            def fetch_head(h):
                k_sb = kv.tile([128, SK], BF16, tag="k", name=f"k{h}")
                nc.sync.dma_start(k_sb, key[h, :, :])
                v_sb = kv.tile([128, JT, HD], pv_dt, tag="v", name=f"v{h}")
                nc.sync.dma_start(v_sb, value[h, :, :, :])
                k_sbs[h] = k_sb
                v_sbs[h] = v_sb

            # DMA_ENGINES is a serial FIFO: hand-interleave the wq stream
            # (just-in-time for P1) with the attention-phase inputs so
            # neither starves the other. Eighth 0 is split in two halves so
            # the very first matmul starts after ~1.5us of transfer.
            w0 = wqp.tile([128, KT, 256], p1_dt, tag="wq", name="wq0")
            wq_sbs.append(w0)
            nc.sync.dma_start(w0[:, 0:4, :], wq[:, 0:4, 0:256])
            nc.sync.dma_start(xT_all[:, 0:2, :], xT[:, 0:2, :])
            nc.sync.dma_start(w0[:, 4:8, :], wq[:, 4:8, 0:256])
            nc.sync.dma_start(xT_all[:, 2:4, :], xT[:, 2:4, :])
            nc.sync.dma_start(xT_all[:, 4:8, :], xT[:, 4:8, :])
            nc.sync.dma_start(w0[:, 8:, :], wq[:, 8:, 0:256])
            nc.sync.dma_start(xT_all[:, 8:12, :], xT[:, 8:12, :])
            nc.sync.dma_start(xT_all[:, 12:16, :], xT[:, 12:16, :])
            nc.sync.dma_start(bq_all, bq[:, :, :])
            nc.sync.dma_start(ones_sb, onesd[:, :, :])
            nc.sync.dma_start(ones1_sb, ones1d[:, :])
            if causal:
                nc.sync.dma_start(maskS_all, maskS[:, :, :])
                nc.sync.dma_start(ident_sb, ident[:, :])
            fetch_head(0)
            fetch_wq(1, nc.sync)
            fetch_head(1)
            fetch_wq(2, nc.sync)
            fetch_wq(3, nc.sync)
            fetch_head(2)
            fetch_wq(4, nc.sync)
            fetch_head(3)
            fetch_wq(5, nc.sync)
            if not causal:
                nc.sync.dma_start(maskT_all[:, 0:8, :], maskT[:, 0:8, :])
            fetch_wq(6, nc.sync)
            fetch_head(4)
            fetch_wq(7, nc.sync)
            if not causal:
                nc.sync.dma_start(maskT_all[:, 8:16, :], maskT[:, 8:16, :])
            nc.sync.dma_start(bpB_all, bpB[:, :])

            # wp streamed as (o-quarter, k-pair) tiles; quarter 0 plus a few
            # of quarter 1 prefetched early (they land during attention)
            wp_sbs = {}
            wp_seq = [(qt, pr) for qt in range(4) for pr in range(KT // 2)]

            def fetch_wp():
                qt, pr = wp_seq[fetch_wp.i]
                fetch_wp.i += 1
                o0 = 512 * qt
                w_sb = wpp.tile([128, 2, 512], p3_dt, tag="wp",
                                name=f"wp{qt}_{pr}")
                nc.sync.dma_start(
                    w_sb, wp[:, 2 * pr:2 * pr + 2, o0:o0 + 512])
                wp_sbs[(qt, pr)] = w_sb

            fetch_wp.i = 0
            for _ in range(16):
                fetch_wp()

            # ---- PSUM plan (8 banks): scores/psq/bcast pool 2x[128,2,512]
            # (4 banks), Z 2x, out 2x. The q-projection PSUM shares the
            # scores pool so P1 o-tiles and early attention heads interleave.
            sc_bufs = 2 if causal else 4
            ps_s = tc.alloc_tile_pool(name="ps_s", bufs=sc_bufs, space="PSUM")
            ps_o = tc.alloc_tile_pool(name="ps_o", bufs=2, space="PSUM")
            ps_z = tc.alloc_tile_pool(name="ps_z", bufs=2, space="PSUM")

            def emit_o_tile(t):
                w_sb = wq_sbs[t // 2]
                osl = slice(128 * (t % 2), 128 * (t % 2) + 128)
                if causal:
                    psq = ps_s.tile([128, 2, 512], F32, tag="s",
                                    name=f"psq{t}")[:, 0, :]
                else:
                    psq = ps_s.tile([128, ROWS], F32, tag="s",
                                    name=f"psq{t}")
                if p1_fp8:
                    for a in range(KT // 2):
                        nc.tensor.matmul(
                            psq, w_sb[:, 2 * a:2 * a + 2, osl],
                            xT_all[:, 2 * a:2 * a + 2, :],
                            start=(a == 0), stop=(a == KT // 2 - 1),
                            perf_mode=DR)
                else:
                    for k in range(KT):
                        nc.tensor.matmul(
                            psq, w_sb[:, k, osl], xT_all[:, k, :],
                            start=(k == 0), stop=(k == KT - 1))
                nc.scalar.activation(
                    qT_all[:, t, :], psq, IDENT,
                    bias=bq_all[:, t, :],
                    scale=(1.0 / WSCALE) if p1_fp8 else 1.0)

            # software pipeline (lag of 6 j-pairs) that crosses head
            # boundaries: pv of head h's last pairs issue after head h+1's
            # first scores pairs, so the PE never waits on the exp chain.
            # causal: j-pair a streams query columns [64a, 512) (64-row
            # granularity is extent-exact and uniform across cores).
            pend = []

            def consume(h, pa, p2, op, zp, v_sb):
                ofs = 64 * pa if causal else 0
                o_dst = op[:, ofs:]
                z_dst = zp[:, ofs:]
                if pv_fp8:
                    js = 2 * pa
                    nc.tensor.matmul(
                        o_dst, v_sb[:, js:js + 2, :], p2,
                        start=(js == 0), stop=(js == JT - 2),
                        perf_mode=DR, skip_group_check=True)
                    nc.tensor.matmul(
                        z_dst, ones_sb, p2,
                        start=(js == 0), stop=(js == JT - 2),
                        perf_mode=DR, skip_group_check=True)
                else:
                    for u in range(2):
                        j = 2 * pa + u
                        nc.tensor.matmul(
                            o_dst, v_sb[:, j, :], p2[:, u, :],
                            start=(j == 0), stop=(j == JT - 1),
                            skip_group_check=True)
                        nc.tensor.matmul(
                            z_dst, ones_sb[:, 0, :], p2[:, u, :],
                            start=(j == 0), stop=(j == JT - 1),
                            skip_group_check=True)
                if pa == JT // 2 - 1:
                    # normalize: attnT_h = op * (1/Z); 1/Z broadcast across
                    # partitions via a rank-1 PE matmul (512 cycles)
                    rc = sm.tile([1, ROWS], BF16, tag="rc", name=f"rc{h}")
                    with nc.allow_low_precision(reason="bf16 reciprocal"):
                        nc.vector.reciprocal(rc, zp)
                    bc = ps_s.tile([128, 2, 512], F32, tag="s",
                                   name=f"bc{h}")
                    nc.tensor.matmul(bc[:, 0, :], ones1_sb, rc,
                                     start=True, stop=True)
                    rb = sm.tile([128, ROWS], BF16, tag="rb", name=f"rb{h}")
                    with nc.allow_low_precision(reason="bf16 bcast"):
                        nc.vector.tensor_copy(rb, bc[:, 0, :])
                    with nc.allow_low_precision(reason="bf16 attn"):
                        nc.vector.tensor_tensor(attnT_all[:, h, :], op, rb,
                                                op=MULT)

            def emit_head(h):
                k_sb, v_sb = k_sbs.pop(h), v_sbs.pop(h)
                zp = ps_z.tile([1, ROWS], F32, tag="z", name=f"z{h}")
                op = ps_o.tile([128, ROWS], F32, tag="o", name=f"o{h}")

                for pa in range(JT // 2):
                    ofs = 64 * pa if causal else 0
                    W = ROWS - ofs
                    p2 = pp.tile([128, 2, W], pv_dt, tag="p", name=f"p{h}_{pa}")
                    if causal:
                        # scores pair in one bank-aligned PSUM tile; causal
                        # mask lands via a tiny identity-stationary matmul on
                        # the 64-col diagonal strip (cols [64a, 64a+64) ==
                        # the first 64 of this pair's window), so exp reads
                        # PSUM directly and the DVE does no mask work at all
                        sc2 = ps_s.tile([128, 2, 512], F32, tag="s",
                                        name=f"sc{h}_{pa}")
                        for u in range(2):
                            j = 2 * pa + u
                            # strip width: even j only needs its 32 diagonal
                            # cols; odd j needs 64 (32 fully-masked + 32 diag)
                            sw = 32 if u == 0 else 64
                            nc.tensor.matmul(sc2[:, u, :W],
                                             k_sb[:, 128 * j:128 * (j + 1)],
                                             qT_all[:, h, ofs:],
                                             start=True, stop=False,
                                             skip_group_check=True)
                            nc.tensor.matmul(sc2[:, u, 0:sw],
                                             ident_sb, maskS_all[:, j, 0:sw],
                                             start=False, stop=True,
                                             skip_group_check=True)
                        nc.scalar.activation(p2, sc2[:, :, :W], EXP,
                                             scale=SCALE)
                    else:
                        t2 = tp.tile([128, 2, W], BF16, tag="t",
                                     name=f"t{h}_{pa}")
                        for u in range(2):
                            j = 2 * pa + u
                            sc = ps_s.tile([128, ROWS], F32, tag="s",
                                           name=f"sc{h}_{j}")
                            nc.tensor.matmul(sc[:, :W],
                                             k_sb[:, 128 * j:128 * (j + 1)],
                                             qT_all[:, h, ofs:],
                                             start=True, stop=True)
                            nc.vector.scalar_tensor_tensor(
                                t2[:, u, :], sc[:, :W], 1.0,
                                maskT_all[:, j, ofs:], MULT, ADD)
                        nc.scalar.activation(p2, t2, EXP, scale=SCALE)
                    pend.append((h, pa, p2, op, zp, v_sb))
                    if len(pend) > 5:
                        consume(*pend.pop(0))
                if h + 5 < NH:
                    fetch_head(h + 5)

            # interleave: early heads run inside P1's DMA-limited stretch
            done_h = 0
            for t in range(KT):
                emit_o_tile(t)
                if t % 2 == 1 and done_h < 7:
                    emit_head(done_h)
                    done_h += 1
            for h in range(done_h, NH):
                emit_head(h)
            while pend:
                consume(*pend.pop(0))

            # release Z banks first: they free at h15's reciprocal, ~1.3us
            # before the rest of the normalize chain, so the P3 accumulators
            # (allocated next, 2 banks) land there and P3 starts earlier
            ps_z.release()

            # ---- phase 3: output projection ----
            # it-tile outer so each [128, 1024] result evicts (and its Y DMA
            # streams out) while the next accumulates; wp half stays resident
            # ps_s/ps_o stay allocated (idle) through P3; ps_y takes the
            # two Z banks, which free earliest in the h15 normalize chain
            ps_y = tc.alloc_tile_pool(name="ps_y", bufs=2, space="PSUM")
            with tc.tile_pool(name="ypo", bufs=3) as ypo:
                for qt in range(4):
                    o0 = 512 * qt
                    for it in range(IT):
                        last = (qt == 3 and it == IT - 1)
                        psy = ps_y.tile([128, 512], F32, tag="y",
                                        name=f"psy{qt}_{it}")
                        # final tile: accumulate+evict in 256-col halves so
                        # the very last eviction overlaps compute
                        for hs, he in ([(0, 256), (256, 512)] if last
                                       else [(0, 512)]):
                            for pr in range(KT // 2):
                                w_sb = wp_sbs[(qt, pr)]
                                for kk in range(0, 2, 2 if p3_fp8 else 1):
                                    k = 2 * pr + kk
                                    if p3_fp8:
                                        att = attnT_all[:, k:k + 2,
                                                        128 * it:
                                                        128 * (it + 1)]
                                        nc.tensor.matmul(
                                            psy[:, hs:he], att,
                                            w_sb[:, :, hs:he],
                                            start=(k == 0),
                                            stop=(k == KT - 2),
                                            perf_mode=DR)
                                    else:
                                        att = attnT_all[:, k,
                                                        128 * it:
                                                        128 * (it + 1)]
                                        nc.tensor.matmul(
                                            psy[:, hs:he], att,
                                            w_sb[:, kk, hs:he],
                                            start=(k == 0),
                                            stop=(k == KT - 1))
                            y_sb = ypo.tile([128, he - hs], F32, tag="ysb",
                                            name=f"y{qt}_{it}_{hs}")
                            nc.vector.scalar_tensor_tensor(
                                y_sb, psy[:, hs:he],
                                (1.0 / WSCALE) if p3_fp8 else 1.0,
                                bpB_all[:, o0 + hs:o0 + he], MULT, ADD)
                            nc.sync.dma_start(
                                Y[128 * it:128 * (it + 1),
                                  o0 + hs:o0 + he], y_sb)
                    # stream quarter qt+2 now that qt's slots are free
                    for _ in range(KT // 2):
                        if fetch_wp.i < len(wp_seq):
                            fetch_wp()
            ps_y.release()
            ps_o.release()
            ps_s.release()
            wpp.release()
            wqp.release()
            sm.release()
            pp.release()
            tp.release()
            kv.release()

    nc.compile()
    return nc


_CACHE = {}


def _get_nc(causal, p1_fp8=False, pv_fp8=False, p3_fp8=False):
    ck = (causal, p1_fp8, pv_fp8, p3_fp8)
    if ck not in _CACHE:
        _CACHE[ck] = build_kernel(causal, p1_fp8, pv_fp8, p3_fp8)
    return _CACHE[ck]


def _is_causal(attention_mask):
    """True if the mask is exactly the standard causal additive mask."""
    m = attention_mask
    if m.shape != (B, 1, SQ, SK):
        return False
    m0 = np.asarray(m[0, 0])
    tri = np.tril(np.ones((SQ, SK), dtype=bool))
    ref = np.where(tri, np.float32(0.0), np.float32(-1e9))
    if not np.array_equal(m0, ref):
        return False
    for b in range(1, B):
        if not np.array_equal(np.asarray(m[b, 0]), m0):
            return False
    return True


def _arr(x, np_dt):
    return np.ascontiguousarray(x.astype(np_dt))


def kernel(hidden_states, key, value, attention_mask, w_q, b_q, w_proj,
           b_proj, _p1_fp8=False, _pv_fp8=False, _p3_fp8=False, _trace=False):
    import ml_dtypes
    BF = ml_dtypes.bfloat16
    F8D = ml_dtypes.float8_e4m3

    hidden_states = np.asarray(hidden_states)
    key = np.asarray(key)
    value = np.asarray(value)
    attention_mask = np.asarray(attention_mask)
    w_q = np.asarray(w_q, dtype=np.float32)
    b_q = np.asarray(b_q, dtype=np.float32)
    w_proj = np.asarray(w_proj, dtype=np.float32)
    b_proj = np.asarray(b_proj, dtype=np.float32)

    causal = _is_causal(attention_mask)
    mask_np = BF if causal else np.float32
    p1_np = F8D if _p1_fp8 else BF
    pv_np = F8D if _pv_fp8 else BF
    p3_np = F8D if _p3_fp8 else BF

    nc = _get_nc(causal, _p1_fp8, _pv_fp8, _p3_fp8)

    # weights arranged [p, a, o] = w.T[a*128+p, o]
    wq_f = w_q.T * (WSCALE if _p1_fp8 else 1.0)
    wq_arr = _arr(wq_f.reshape(KT, 128, H).transpose(1, 0, 2), p1_np)
    wp_f = w_proj.T * (WSCALE if _p3_fp8 else 1.0)
    wp_arr = _arr(wp_f.reshape(KT, 128, H).transpose(1, 0, 2), p3_np)
    bq_arr = _arr(b_q.reshape(KT, 128).T[:, :, None], np.float32)
    bp_arr = _arr(np.broadcast_to(b_proj[None, :], (128, H)), np.float32)
    ones_arr = np.ones((128, 2, 1), dtype=pv_np)
    inv_scale = np.float32(1.0 / SCALE)

    key_b = [_arr(key[b * NH:(b + 1) * NH], BF) for b in range(B)]
    # value arranged [h, p, j, d] = value[b, h, j*128+p, d]
    val_b = [
        _arr(value[b].reshape(NH, JT, 128, HD).transpose(0, 2, 1, 3), pv_np)
        for b in range(B)
    ]

    def core_rows(c):
        b, cc = c // 4, c % 4
        if causal:
            return b, cc + 4 * np.arange(ROWS)
        return b, np.arange(ROWS * cc, ROWS * cc + ROWS)

    in_maps = []
    for c in range(NCORES):
        b, rows = core_rows(c)
        xT_c = hidden_states[b][rows, :].T  # [H, ROWS]
        xT_arr = _arr(xT_c.reshape(KT, 128, ROWS).transpose(1, 0, 2), p1_np)
        m_c = (attention_mask[b, 0][rows, :].T * inv_scale)  # [SK, ROWS]
        im = dict(
            xT=xT_arr, wq=wq_arr, bq=bq_arr, key=key_b[b], value=val_b[b],
            wp=wp_arr, bpB=bp_arr, onesd=ones_arr,
            ones1d=np.ones((1, 128), dtype=BF),
        )
        if causal:
            # 64-col diagonal strips: maskS[p, j, c] = m_c[128j+p, 64(j//2)+c]
            mS = np.empty((128, JT, 64), dtype=np.float32)
            for j in range(JT):
                c0 = 64 * (j // 2)
                mS[:, j, :] = m_c[128 * j:128 * (j + 1), c0:c0 + 64]
            im["maskS"] = _arr(mS, BF)
            im["ident"] = _arr(np.eye(128, dtype=np.float32), BF)
        else:
            im["maskT"] = _arr(
                m_c.reshape(JT, 128, ROWS).transpose(1, 0, 2), mask_np)
        in_maps.append(im)

    kw = {}
    if _trace:
        kw = dict(trace=True, trace_cores=list(range(NCORES)),
                  stitch_traces=False)
    res = run_bass_kernel_spmd(nc, in_maps, core_ids=list(range(NCORES)), **kw)
    if _trace:
        kernel._last_result = res

    out = np.empty((B, SQ, H), dtype=np.float32)
    for c in range(NCORES):
        b, rows = core_rows(c)
        out[b][rows, :] = res.results[c]["Y"]
    return out


if __name__ == "__main__":
    pass

